# revision 39
# baseline (speedup 1.0000x reference)
"""Causal self-attention + depthwise-conv + out-proj fused TRN2 kernel.

Model (B=4, T=2048, C=1024, H=16, D=64, conv K=4):
    qkv = x @ W_qkv.T ; causal softmax attention per head ;
    y2 = attn + causal_depthwise_conv(attn) + conv_b ; out = y2 @ W_out.T

Sharding over 8 NeuronCores: core c -> (batch b = c//2, head-group g = c%2).
Each core computes q/k/v for its 8 heads (fp32r matmuls against x[b].T),
bf16 flash-style causal attention in transposed [d, t] layout (exp softmax
without max subtraction - logits are O(1)), the depthwise conv as diagonal
matmuls along the channel partition with the residual folded into the
center tap, then a pairwise AllGather of the 512-channel activation and
half of the output projection columns.

Layout notes:
  - scores are computed transposed: S^T[k, q] = K^T.T @ Q^T so that the AV
    matmul can consume exp(S^T) directly as the moving operand.
  - causal masking is done by pre-filling the diagonal psum strip with a
    {0, -30000} staircase via an identity-stationary matmul (start=True),
    then accumulating the scores on top (start=False).
  - the AV stationary is [V_h | ones]: rows 0-63 of the psum get attn^T,
    rows 64-127 get 64 replicas of the softmax denominator, so the
    normalization is a reciprocal + lane-wise multiply.
"""

import numpy as np
import ml_dtypes

import concourse.bacc as bacc
import concourse.mybir as mybir
import concourse.tile as tile
from concourse.bass_utils import run_bass_kernel_spmd

F32R = mybir.dt.float32r
F32 = mybir.dt.float32
BF16 = mybir.dt.bfloat16

B, T, C, H, D, K = 4, 2048, 1024, 16, 64, 4
HC = H // 2  # heads per core (8)
CC = C // 2  # channels per core (512)
NEG = -30000.0
NCORES = 8
REPLICA_GROUPS = [[0, 1], [2, 3], [4, 5], [6, 7]]
NTB = T // 512  # 512-wide t blocks (4)
NTT = T // 128  # 128-wide t tiles (16)
NCT = C // 128  # 128-wide input-channel tiles (8)
NPAIR = 4  # head pairs per core

_NC_CACHE = {}


def build(debug=False, reps=1, qkv_dt=BF16, collective=True, stage1=False):
    nc = bacc.Bacc(None, num_devices=NCORES)

    xT_d = nc.dram_tensor("xT", [C, T], qkv_dt, kind="ExternalInput")
    wqk_d = nc.dram_tensor("wqk", [C, 1024], qkv_dt, kind="ExternalInput")
    wv_d = nc.dram_tensor("wv", [C, CC], qkv_dt, kind="ExternalInput")
    wout_d = nc.dram_tensor("wout", [C, CC], BF16, kind="ExternalInput")
    maskdiag_d = nc.dram_tensor("maskdiag", [128, 512], BF16,
                                kind="ExternalInput")
    convw_d = nc.dram_tensor("convw", [128, NPAIR, K + 1], F32,
                             kind="ExternalInput")
    outT_d = nc.dram_tensor("outT", [CC, T], BF16, kind="ExternalOutput")
    if debug:
        dbg_qT = nc.dram_tensor("dbg_qT", [128, T], F32, kind="ExternalOutput")
        dbg_kT = nc.dram_tensor("dbg_kT", [128, T], F32, kind="ExternalOutput")
        dbg_yt = nc.dram_tensor("dbg_yt", [128, T], F32, kind="ExternalOutput")
        dbg_vo = nc.dram_tensor("dbg_vo", [128, 2, 128], F32, kind="ExternalOutput")
        dbg_pt = nc.dram_tensor("dbg_pt", [128, 1024], F32, kind="ExternalOutput")
        dbg_att = nc.dram_tensor("dbg_att", [128, 512], F32, kind="ExternalOutput")

    with tile.TileContext(nc) as tc:
        with (
            tc.tile_pool(name="consts", bufs=1) as consts,
            tc.tile_pool(name="work", bufs=2) as work,
            tc.tile_pool(name="ps_st", bufs=2, space="PSUM") as ps_st,
            tc.tile_pool(name="ps_att", bufs=1, space="PSUM") as ps_att,
            tc.tile_pool(name="ps_mm", bufs=2, space="PSUM") as ps_mm,
            tc.tile_pool(name="dram", bufs=1, space="DRAM") as dram,
        ):
            # ---------- constants / big loads ----------
            xT = consts.tile([128, NCT, T], xT_d.dtype, tag="xT")
            xT_r = xT_d.rearrange("(n p) m -> p n m", p=128)
            for ct in range(NCT):
                nc.sync.dma_start(xT[:, ct, :], xT_r[:, ct, :])
            maskdiag = consts.tile([128, 2, 256], BF16, tag="maskdiag")
            nc.sync.dma_start(maskdiag[:], maskdiag_d.rearrange("p (a b) -> p a b", a=2))
            convw = consts.tile([128, NPAIR, K + 1], F32, tag="convw")
            nc.sync.dma_start(convw[:], convw_d[:])
            for rep in range(reps):
                _emit_body(nc, tc, consts, work, ps_st, ps_att, ps_mm, dram,
                           locals(), debug and rep == 0, rep, collective, stage1)

    nc.compile()
    return nc


def _emit_body(nc, tc, consts, work, ps_st, ps_att, ps_mm, dram, env, debug, rep,
               collective=True, stage1=False):
    xT = env["xT"]
    maskdiag = env["maskdiag"]
    convw = env["convw"]
    consts_pool = consts
    wv_d = env["wv_d"]
    wqk_d = env["wqk_d"]
    wout_d = env["wout_d"]
    outT_d = env["outT_d"]
    if debug:
        dbg_qT = env["dbg_qT"]
        dbg_kT = env["dbg_kT"]
        dbg_yt = env["dbg_yt"]
        dbg_vo = env["dbg_vo"]
        dbg_pt = env["dbg_pt"]
        dbg_att = env["dbg_att"]
    R = f"r{rep}_"

    # V with embedded ones columns per head: head block j is
    # [ones(64) | v_j(64)] so the AV stationary is one contiguous 128-col AP.
    vsb = consts_pool.tile([128, NTT, HC, 128], BF16, tag="vsb", bufs=1,
                           name="vsb")
    if rep == 0:
        nc.vector.memset(vsb[:, :, :, 0:64], 1.0)

    # wv shares its slot with wout (wv is dead once V is computed)
    wv = consts.tile([128, NCT, CC], wv_d.dtype, tag="w2", bufs=2,
                 name=R + "wv")
    wv_r = wv_d.rearrange("(n p) m -> p n m", p=128)
    for ct in range(NCT):
        nc.sync.dma_start(wv[:, ct, :], wv_r[:, ct, :])

    if True:
        if True:
            # ---------- V projection into vsb blocks 1..8 ----------
            for tt in range(NTT):
                vps = ps_mm.tile([128, 512], F32, tag="mm", name=f"vps{tt}")
                for ct in range(NCT):
                    nc.tensor.matmul(
                        vps[:],
                        xT[:, ct, tt * 128 : tt * 128 + 128],
                        wv[:, ct, :],
                        start=(ct == 0),
                        stop=(ct == NCT - 1),
                    )
                nc.vector.tensor_copy(vsb[:, tt, :, 64:128], vps[:])

            # gathered-order W_out, loaded into the slot wv vacates
            wout = consts.tile([128, NCT, CC], BF16, tag="w2", bufs=2,
                   name=R + "wout")
            nc.sync.dma_start(wout[:], wout_d.rearrange("(n p) m -> p n m", p=128))

            y2g_tiles = []
            for p in range(NPAIR):
                # ---------- QK projection for head pair p ----------
                wqk = work.tile([128, NCT, 256], wqk_d.dtype, tag="wqk", bufs=2,
                                name=f"wqk{p}")
                nc.sync.dma_start(
                    wqk[:],
                    wqk_d[:, 256 * p : 256 * p + 256].rearrange(
                        "(n p) m -> p n m", p=128
                    ),
                )
                qT = work.tile([128, T], BF16, tag="qT", bufs=4, name=f"qT{p}")
                kT = work.tile([128, T], BF16, tag="kT", bufs=4, name=f"kT{p}")
                for fs, dst in ((0, qT), (1, kT)):
                    for tb in range(NTB):
                        ps = ps_mm.tile([128, 512], F32, tag="mm",
                                        name=f"qkps{p}_{fs}_{tb}")
                        for ct in range(NCT):
                            nc.tensor.matmul(
                                ps[:],
                                wqk[:, ct, 128 * fs : 128 * fs + 128],
                                xT[:, ct, 512 * tb : 512 * tb + 512],
                                start=(ct == 0),
                                stop=(ct == NCT - 1),
                            )
                        nc.vector.tensor_copy(
                            dst[:, 512 * tb : 512 * tb + 512], ps[:]
                        )

                if debug and p == 0:
                    for nm, src, dst in (("dq", qT, dbg_qT), ("dk", kT, dbg_kT)):
                        dtile = work.tile([128, T], F32, tag="dbg", bufs=1,
                                          name=f"dbg{nm}")
                        nc.vector.tensor_copy(dtile[:], src[:])
                        nc.sync.dma_start(dst[:], dtile[:])

                if stage1 and p == 2:
                    # ---------- out-proj stage 1: pairs {0,1} ----------
                    # emitted after pair-2's QK proj so its psum-ring slots
                    # don't gate that; runs in PE gaps of ACT-paced attention
                    osba = work.tile([128, NTB, 4, 512], F32, tag="osba",
                                     bufs=1, name="osba")
                    for tb in range(NTB):
                        t0 = 512 * tb
                        ysbs = _load_ysb(nc, work, y2g_tiles, tb, t0, 0, 4)
                        for ot in range(4):
                            ops_ = ps_mm.tile([128, 512], F32, tag="mm",
                                              name=f"opsA{tb}_{ot}")
                            for gs in range(4):
                                nc.tensor.matmul(
                                    ops_[:],
                                    wout[:, gs, 128 * ot : 128 * ot + 128],
                                    ysbs[gs][:],
                                    start=(gs == 0),
                                    stop=(gs == 3),
                                )
                            nc.vector.tensor_copy(osba[:, tb, ot, :], ops_[:])

                # ---------- attention for the two heads of pair p ----------
                yt = work.tile([128, T], BF16, tag="yt", bufs=2, name=f"yt{p}")
                for qb in range(NTB):
                    q0 = 512 * qb
                    att = ps_att.tile([128, 2, 512], F32, tag="att", bufs=1,
                                      name=f"att{p}_{qb}")
                    ngrp = 2 * qb + 2
                    for grp in range(ngrp):
                        # group-level start column (w0 of the first half);
                        # half 1 computes 128 garbage cols left of its own
                        # diagonal so the exp AP below stays rectangular
                        w0g = max(0, 128 * (2 * grp - 4 * qb))
                        for h in range(2):
                            hp = 64 * h
                            st = ps_st.tile([128, 2, 512], F32, tag="st", bufs=2,
                                            name=f"st{p}_{qb}_{grp}_{h}")
                            for half in range(2):
                                kt = 2 * grp + half
                                nc.tensor.matmul(
                                    st[:, half, w0g:512],
                                    kT[hp : hp + 64, 128 * kt : 128 * kt + 128],
                                    qT[hp : hp + 64, q0 + w0g : q0 + 512],
                                    start=True,
                                    stop=True,
                                )
                            pt = work.tile([128, 2, 512], BF16, tag="pt", bufs=8,
                                           name=f"pt{p}_{qb}_{grp}_{h}")
                            nc.scalar.activation(
                                out=pt[:, :, w0g:512],
                                in_=st[:, :, w0g:512],
                                func=mybir.ActivationFunctionType.Exp,
                                scale=0.125,
                            )
                            if grp >= 2 * qb:
                                # diagonal group: zero the masked triangles
                                # (plus half1's dead lead-in) in one op
                                nc.vector.tensor_mul(
                                    out=pt[:, :, w0g : w0g + 256],
                                    in0=pt[:, :, w0g : w0g + 256],
                                    in1=maskdiag[:],
                                )
                            for half in range(2):
                                kt = 2 * grp + half
                                w0 = max(0, 128 * (kt - 4 * qb))
                                nc.tensor.matmul(
                                    att[:, h, w0:512],
                                    vsb[:, kt, 2 * p + h, :],
                                    pt[:, half, w0:512],
                                    start=(kt == 0),
                                    stop=(kt == 4 * qb + 3),
                                )
                    rec = work.tile([64, 2, 512], F32, tag="rec", bufs=2,
                                    name=f"rec{p}_{qb}")
                    nc.vector.reciprocal_approx_fast(rec[:], att[0:64, :, :])
                    for h in range(2):
                        nc.vector.tensor_mul(
                            out=yt[64 * h : 64 * h + 64, q0 : q0 + 512],
                            in0=att[64:128, h, :],
                            in1=rec[:, h, :],
                        )

                if debug and p == 0:
                    dyt = work.tile([128, T], F32, tag="dbg", bufs=1,
                                    name="dbgyt")
                    nc.vector.tensor_copy(dyt[:], yt[:])
                    nc.sync.dma_start(dbg_yt[:], dyt[:])

                # ---------- depthwise causal conv + residual + bias (DVE) ----
                # deprioritized so the attention-critical DVE ops (mask-mult,
                # reciprocal) of the next pair aren't queued behind it
                y2loc = dram.tile([128, T], BF16, tag=R + f"y2loc{p}",
                                  name=R + f"y2loc{p}")
                for tb in range(NTB):
                    t0 = 512 * tb
                    y2sb = work.tile([128, 512], BF16, tag="y2sb", bufs=4,
                                     name=f"y2sb{p}_{tb}")
                    # lag-0 tap (residual folded in) + bias
                    nc.vector.tensor_scalar(
                        out=y2sb[:],
                        in0=yt[:, t0 : t0 + 512],
                        scalar1=convw[:, p, 3:4],
                        scalar2=convw[:, p, 4:5],
                        op0=mybir.AluOpType.mult,
                        op1=mybir.AluOpType.add,
                    )
                    for lag in range(1, 4):
                        j = 3 - lag
                        lo = max(0, lag - t0)
                        nc.vector.scalar_tensor_tensor(
                            out=y2sb[:, lo:512],
                            in0=yt[:, t0 + lo - lag : t0 + 512 - lag],
                            scalar=convw[:, p, j : j + 1],
                            in1=y2sb[:, lo:512],
                            op0=mybir.AluOpType.mult,
                            op1=mybir.AluOpType.add,
                        )
                    nc.gpsimd.dma_start(y2loc[:, t0 : t0 + 512], y2sb[:])

                # ---------- pairwise AllGather of this 128-channel slab ----------
                y2g = dram.tile([256, T], BF16, tag=R + f"y2g{p}",
                                name=R + f"y2g{p}")
                if collective:
                    nc.gpsimd.collective_compute(
                        "AllGather",
                        mybir.AluOpType.bypass,
                        replica_groups=REPLICA_GROUPS,
                        ins=[y2loc.opt()],
                        outs=[y2g.opt()],
                    )
                else:
                    # sim stand-in: same DRAM traffic shape, no collective
                    nc.gpsimd.dma_start(y2g[0:128, :], y2loc[:])
                    nc.gpsimd.dma_start(y2g[128:256, :], y2loc[:])
                y2g_tiles.append(y2g)

            # ---------- out-proj stage 2: pairs {2,3} + stage-1 partials ----
            gA = 4 if stage1 else 0
            for tb in range(NTB):
                t0 = 512 * tb
                ysbs = _load_ysb(nc, work, y2g_tiles, tb, t0, gA, 8)
                for ot in range(4):
                    ops_ = ps_mm.tile([128, 512], F32, tag="mm",
                                      name=f"opsB{tb}_{ot}")
                    for gs in range(gA, 8):
                        nc.tensor.matmul(
                            ops_[:],
                            wout[:, gs, 128 * ot : 128 * ot + 128],
                            ysbs[gs - gA][:],
                            start=(gs == gA),
                            stop=(gs == 7),
                        )
                    osb = work.tile([128, 512], BF16, tag="osb", bufs=4,
                                    name=f"osb{tb}_{ot}")
                    if stage1:
                        nc.vector.tensor_tensor(
                            out=osb[:],
                            in0=ops_[:],
                            in1=osba[:, tb, ot, :],
                            op=mybir.AluOpType.add,
                        )
                    else:
                        nc.vector.tensor_copy(osb[:], ops_[:])
                    nc.gpsimd.dma_start(
                        outT_d[128 * ot : 128 * ot + 128, t0 : t0 + 512], osb[:]
                    )



def _load_ysb(nc, work, y2g_tiles, tb, t0, g0, g1):
    ysbs = []
    for gs in range(g0, g1):
        p, parity = gs // 2, gs % 2
        ysb = work.tile([128, 512], BF16, tag=f"ysb{gs}", bufs=2,
                        name=f"ysb{tb}_{gs}")
        nc.sync.dma_start(
            ysb[:],
            y2g_tiles[p][128 * parity : 128 * parity + 128, t0 : t0 + 512],
        )
        ysbs.append(ysb)
    return ysbs


def _make_maskdiag():
    # [128, 2, 256] flattened to [128, 512]:
    # block 0 (half 0): [keep-triangle(128) | ones(128)]
    # block 1 (half 1): [zeros(128)         | keep-triangle(128)]
    kp = np.arange(128)[:, None]
    col = np.arange(128)[None, :]
    tri = (kp <= col).astype(np.float32)
    m = np.zeros((128, 512), np.float32)
    m[:, 0:128] = tri
    m[:, 128:256] = 1.0
    m[:, 256:384] = 0.0
    m[:, 384:512] = tri
    return m.astype(ml_dtypes.bfloat16)


def prepare_in_maps(x, W_qkv, W_out, conv_w, conv_b, qkv_np=ml_dtypes.bfloat16):
    x = np.asarray(x, np.float32)
    W_qkv = np.asarray(W_qkv, np.float32)
    W_out = np.asarray(W_out, np.float32)
    conv_w = np.asarray(conv_w, np.float32).reshape(C, K)
    conv_b = np.asarray(conv_b, np.float32)

    maskdiag = _make_maskdiag()

    # gathered channel order: row r of y2g stack -> global channel
    perm = np.empty(C, np.int64)
    for r in range(C):
        p, parity, within = r // 256, (r % 256) // 128, r % 128
        perm[r] = 512 * parity + 128 * p + within

    in_maps = []
    for core in range(NCORES):
        b, g = core // 2, core % 2
        xT = np.ascontiguousarray(x[b].T)  # [C, T]
        # wqk: cols [256p:256p+128] = q rows of pair p (.T), then k rows
        wqk = np.empty((C, 1024), np.float32)
        for p in range(NPAIR):
            r0 = 64 * (8 * g + 2 * p)
            wqk[:, 256 * p : 256 * p + 128] = W_qkv[r0 : r0 + 128, :].T
            wqk[:, 256 * p + 128 : 256 * p + 256] = W_qkv[
                1024 + r0 : 1024 + r0 + 128, :
            ].T
        wv = np.ascontiguousarray(W_qkv[2048 + CC * g : 2048 + CC * g + CC, :].T)
        # W_out columns for this core's output slice, rows in gathered order
        wout = np.ascontiguousarray(
            W_out[CC * g : CC * g + CC, :].T[perm, :]
        ).astype(ml_dtypes.bfloat16)
        # conv taps + bias, per-partition layout: [128, pair, (w0..w3, bias)]
        convw = np.zeros((128, NPAIR, K + 1), np.float32)
        for p in range(NPAIR):
            ch0 = CC * g + 128 * p
            for j in range(K):
                w = conv_w[ch0 : ch0 + 128, j]
                if j == K - 1:
                    w = w + 1.0  # residual folded into the lag-0 tap
                convw[:, p, j] = w
            convw[:, p, K] = conv_b[ch0 : ch0 + 128]
        in_maps.append(
            {
                "xT": xT.astype(qkv_np),
                "wqk": wqk.astype(qkv_np),
                "wv": wv.astype(qkv_np),
                "wout": wout,
                "maskdiag": maskdiag,
                "convw": convw,
            }
        )
    return in_maps


def assemble_output(results):
    out = np.empty((B, T, C), np.float32)
    for core in range(NCORES):
        b, g = core // 2, core % 2
        outT = np.asarray(results[core]["outT"], np.float32)  # [CC, T]
        out[b, :, CC * g : CC * g + CC] = outT.T
    return out


def kernel(x, W_qkv, W_out, conv_w, conv_b):
    if "nc" not in _NC_CACHE:
        _NC_CACHE["nc"] = build()
    nc = _NC_CACHE["nc"]
    in_maps = prepare_in_maps(x, W_qkv, W_out, conv_w, conv_b)
    res = run_bass_kernel_spmd(nc, in_maps, list(range(NCORES)))
    return assemble_output(res.results)

